# revision 3
# baseline (speedup 1.0000x reference)
"""Trainium2 Bass kernel for the LogicLayer (difflogic) problem — v2.

out[i, o] = c0[o] + ca[o]*a + cb[o]*b + cab[o]*a*b
  with a = x[i, idx_a[o]], b = x[i, idx_b[o]],
  [c0, ca, cb, cab] = softmax(weights[o]) @ GATE_COEFFS.

Strategy (8 cores, OUTPUT-sharded, 1024 out cols/core), output-major:
  - x transposed on host to xt [IN_DIM, BATCH] fp16, stays in HBM.
  - dma_gather pulls full batch rows xt[idx_a[o], :] straight from HBM
    into SBUF tiles [128 outs, j, BATCH] (one 8KB row per descriptor,
    ~360GB/s on the 16 DMA engines; descriptor gen on gpsimd is ~1us).
  - coefficients become per-partition scalars: DVE tensor_scalar +
    tensor_tensor and ACT Identity compute
      out = (b*cab + ca)*a + (b*cb + c0)
    4 ops/elem, fp16, fully overlapped under the DMA traffic.
  - store y fp16 [chunk, 128, j, BATCH]; host reassembles + transposes.
"""

import numpy as np

BATCH, IN_DIM, OUT_DIM = 4096, 8192, 8192
N_CORES = 8
OC = OUT_DIM // N_CORES          # 1024 out cols per core
CHUNK = 256                      # out cols per chunk
NCH = OC // CHUNK                # 4 chunks
NJ = CHUNK // 128                # 2 partition-blocks per chunk

GATE_COEFFS = np.array([
    [0, 0, 0, 0], [0, 0, 0, 1], [0, 1, 0, -1], [0, 1, 0, 0],
    [0, 0, 1, -1], [0, 0, 1, 0], [0, 1, 1, -2], [0, 1, 1, -1],
    [1, -1, -1, 1], [1, -1, -1, 2], [1, 0, -1, 0], [1, 0, -1, 1],
    [1, -1, 0, 0], [1, -1, 0, 1], [1, 0, 0, -1], [1, 0, 0, 0],
], dtype=np.float32)  # [16, 4]

_CACHE = {}


def _build_nc(n_reps=1):
    import concourse.bacc as bacc
    import concourse.mybir as mybir

    f16 = mybir.dt.float16
    f32 = mybir.dt.float32
    i16 = mybir.dt.int16
    Alu = mybir.AluOpType
    ActFn = mybir.ActivationFunctionType

    nc = bacc.Bacc("TRN2", target_bir_lowering=False, debug=False,
                   num_devices=N_CORES)
    xt = nc.dram_tensor("xt", [IN_DIM, BATCH], f16, kind="ExternalInput")
    idxw = nc.dram_tensor("idxw", [128, NCH * 2 * (CHUNK // 16)], i16,
                          kind="ExternalInput")
    coefw = nc.dram_tensor("coefw", [128, NCH * NJ * 4], f32,
                           kind="ExternalInput")
    y = nc.dram_tensor("y", [NCH, 128, NJ, BATCH], f16,
                       kind="ExternalOutput")

    IC = CHUNK // 16  # idx cols per gather (16)

    with (
        nc.Block() as block,
        nc.sbuf_tensor("ab_sb", [128, 2, 2 * NJ, BATCH], f16) as ab_sb,
        nc.sbuf_tensor("o_sb", [128, 2, NJ, BATCH], f16) as o_sb,
        nc.sbuf_tensor("u_sb", [128, BATCH], f16) as u_sb,
        nc.sbuf_tensor("v_sb", [128, 2, NJ, BATCH], f16) as v_sb,
        nc.sbuf_tensor("idx_sb", [128, NCH * 2 * IC], i16) as idx_sb,
        nc.sbuf_tensor("coef_sb", [128, NCH * NJ * 4], f32) as coef_sb,
        nc.semaphore("lsem") as lsem,
        nc.semaphore("csem") as csem,
        nc.semaphore("gs0") as gs0,
        nc.semaphore("gs1") as gs1,
        nc.semaphore("vch") as vch,
        nc.semaphore("vprod") as vprod,
        nc.semaphore("vdone") as vdone,
        nc.semaphore("ssem") as ssem,
    ):
        T = n_reps * NCH  # total chunks
        gsems = [gs0, gs1]

        def gwait(eng, t):
            # chunk t's gather complete (parity sem: at most two chunks'
            # gathers in flight, so counts never mix)
            eng.wait_ge(gsems[t % 2], 16 * (t // 2 + 1))

        def cf(t4, j, k):  # scalar AP for coeff k of (chunk, j)
            col = (t4 * NJ + j) * 4 + k
            return coef_sb[:, col:col + 1]

        @block.gpsimd
        def _(gpsimd):
            gpsimd.dma_start(idx_sb[:], idxw.ap()).then_inc(lsem, 16)
            gpsimd.dma_start(coef_sb[:], coefw.ap()).then_inc(csem, 16)
            gpsimd.wait_ge(lsem, 16)
            for t in range(T):
                t4 = t % NCH
                if t >= 2:
                    gpsimd.wait_ge(vdone, t - 1)
                    gpsimd.wait_ge(vprod, NJ * (t - 1))
                k = t % 2
                iab = idx_sb[:, (t4 * 2) * IC:(t4 * 2 + 2) * IC]
                gpsimd.dma_gather(
                    ab_sb[:, k], xt.ap(), iab, 2 * CHUNK, 2 * CHUNK, BATCH,
                ).then_inc(gsems[t % 2], 16)

        @block.scalar
        def _(scalar):
            scalar.wait_ge(csem, 16)
            for t in range(T):
                t4 = t % NCH
                k = t % 2
                gwait(scalar, t)
                if t >= 2:
                    scalar.wait_ge(vdone, t - 1)
                for j in range(NJ):
                    # v = b*cb + c0
                    scalar.activation(
                        v_sb[:, k, j], ab_sb[:, k, NJ + j], ActFn.Identity,
                        bias=cf(t4, j, 0), scale=cf(t4, j, 2),
                    ).then_inc(vprod, 1)

        @block.vector
        def _(vector):
            vector.wait_ge(csem, 16)
            nv = 0
            for t in range(T):
                t4 = t % NCH
                k = t % 2
                gwait(vector, t)
                if t >= 2:
                    vector.wait_ge(ssem, 16 * (t - 1))
                for j in range(NJ):
                    a = ab_sb[:, k, j]
                    b = ab_sb[:, k, NJ + j]
                    # u = b*cab + ca
                    vector.tensor_scalar(
                        u_sb[:], b, cf(t4, j, 3), cf(t4, j, 1),
                        Alu.mult, Alu.add,
                    ).then_inc(vch, 1)
                    nv += 1
                    vector.wait_ge(vch, nv)
                    # u = u * a
                    vector.tensor_tensor(
                        u_sb[:], u_sb[:], a, Alu.mult,
                    ).then_inc(vch, 1)
                    nv += 1
                    vector.wait_ge(vch, nv)
                    vector.wait_ge(vprod, NJ * t + j + 1)
                    # out = u + v
                    inst = vector.tensor_tensor(
                        o_sb[:, k, j], u_sb[:], v_sb[:, k, j], Alu.add,
                    )
                    if j == NJ - 1:
                        inst.then_inc(vdone, 1)
                    else:
                        inst.then_inc(vch, 1)
                        nv += 1

        @block.sync
        def _(sync):
            for t in range(T):
                t4 = t % NCH
                k = t % 2
                sync.wait_ge(vdone, t + 1)
                sync.dma_start(y.ap()[t4], o_sb[:, k]).then_inc(ssem, 16)
            sync.wait_ge(ssem, 16 * T)

    nc.compile()
    return nc


def _build_nc_loop(n_reps):
    """Hardware-looped variant (constant program size, n_reps of work).

    Used only for timing: rep 0 is peeled (unrolled, as in _build_nc),
    reps 1..n_reps run in a per-engine Fori loop with register-computed
    semaphore targets. Structure per rep is identical to _build_nc.
    """
    import concourse.bacc as bacc
    import concourse.mybir as mybir

    f16 = mybir.dt.float16
    f32 = mybir.dt.float32
    i16 = mybir.dt.int16
    Alu = mybir.AluOpType
    ActFn = mybir.ActivationFunctionType

    assert n_reps >= 2 and 16 * NCH * n_reps < 60000

    nc = bacc.Bacc("TRN2", target_bir_lowering=False, debug=False,
                   num_devices=N_CORES)
    xt = nc.dram_tensor("xt", [IN_DIM, BATCH], f16, kind="ExternalInput")
    idxw = nc.dram_tensor("idxw", [128, NCH * 2 * (CHUNK // 16)], i16,
                          kind="ExternalInput")
    coefw = nc.dram_tensor("coefw", [128, NCH * NJ * 4], f32,
                           kind="ExternalInput")
    y = nc.dram_tensor("y", [NCH, 128, NJ, BATCH], f16,
                       kind="ExternalOutput")

    IC = CHUNK // 16

    with (
        nc.Block() as block,
        nc.sbuf_tensor("ab_sb", [128, 2, 2 * NJ, BATCH], f16) as ab_sb,
        nc.sbuf_tensor("o_sb", [128, 2, NJ, BATCH], f16) as o_sb,
        nc.sbuf_tensor("u_sb", [128, BATCH], f16) as u_sb,
        nc.sbuf_tensor("v_sb", [128, 2, NJ, BATCH], f16) as v_sb,
        nc.sbuf_tensor("idx_sb", [128, NCH * 2 * IC], i16) as idx_sb,
        nc.sbuf_tensor("coef_sb", [128, NCH * NJ * 4], f32) as coef_sb,
        nc.semaphore("lsem") as lsem,
        nc.semaphore("csem") as csem,
        nc.semaphore("gs0") as gs0,
        nc.semaphore("gs1") as gs1,
        nc.semaphore("vch") as vch,
        nc.semaphore("vprod") as vprod,
        nc.semaphore("vdone") as vdone,
        nc.semaphore("ssem") as ssem,
    ):
        gsems = [gs0, gs1]

        def cf(t4, j, k):
            col = (t4 * NJ + j) * 4 + k
            return coef_sb[:, col:col + 1]

        def pool_gathers(gpsimd, t4):
            iab = idx_sb[:, (t4 * 2) * IC:(t4 * 2 + 2) * IC]
            gpsimd.dma_gather(
                ab_sb[:, t4 % 2], xt.ap(), iab, 2 * CHUNK, 2 * CHUNK, BATCH,
            ).then_inc(gsems[t4 % 2], 16)

        @block.gpsimd
        def _(gpsimd):
            gpsimd.dma_start(idx_sb[:], idxw.ap()).then_inc(lsem, 16)
            gpsimd.dma_start(coef_sb[:], coefw.ap()).then_inc(csem, 16)
            gpsimd.wait_ge(lsem, 16)
            for t4 in range(NCH):  # peeled rep 0
                if t4 >= 2:
                    gpsimd.wait_ge(vdone, t4 - 1)
                    gpsimd.wait_ge(vprod, NJ * (t4 - 1))
                pool_gathers(gpsimd, t4)
            with gpsimd.Fori(1, n_reps) as i:
                for t4 in range(NCH):
                    gpsimd.wait_ge(vdone, i * NCH + t4 - 1)
                    gpsimd.wait_ge(vprod, i * (NJ * NCH) + NJ * t4 - NJ)
                    pool_gathers(gpsimd, t4)

        @block.scalar
        def _(scalar):
            scalar.wait_ge(csem, 16)

            def act_chunk(t4, gt, vt):
                scalar.wait_ge(gsems[t4 % 2], gt)
                if vt is not None:
                    scalar.wait_ge(vdone, vt)
                for j in range(NJ):
                    scalar.activation(
                        v_sb[:, t4 % 2, j], ab_sb[:, t4 % 2, NJ + j],
                        ActFn.Identity,
                        bias=cf(t4, j, 0), scale=cf(t4, j, 2),
                    ).then_inc(vprod, 1)

            for t4 in range(NCH):  # peeled rep 0
                act_chunk(t4, 16 * (t4 // 2 + 1),
                          t4 - 1 if t4 >= 2 else None)
            with scalar.Fori(1, n_reps) as i:
                for t4 in range(NCH):
                    act_chunk(t4, i * 32 + 16 * (t4 // 2) + 16,
                              i * NCH + t4 - 1)

        @block.vector
        def _(vector):
            vector.wait_ge(csem, 16)
            VPC = 2 * NJ + (NJ - 1)  # vch incs per chunk (5)

            def dve_chunk(t4, gt, st, nv0, vp0):
                vector.wait_ge(gsems[t4 % 2], gt)
                if st is not None:
                    vector.wait_ge(ssem, st)
                nvo = 0
                for j in range(NJ):
                    a = ab_sb[:, t4 % 2, j]
                    b = ab_sb[:, t4 % 2, NJ + j]
                    vector.tensor_scalar(
                        u_sb[:], b, cf(t4, j, 3), cf(t4, j, 1),
                        Alu.mult, Alu.add,
                    ).then_inc(vch, 1)
                    nvo += 1
                    vector.wait_ge(vch, nv0 + nvo)
                    vector.tensor_tensor(
                        u_sb[:], u_sb[:], a, Alu.mult,
                    ).then_inc(vch, 1)
                    nvo += 1
                    vector.wait_ge(vch, nv0 + nvo)
                    vector.wait_ge(vprod, vp0 + j + 1)
                    inst = vector.tensor_tensor(
                        o_sb[:, t4 % 2, j], u_sb[:], v_sb[:, t4 % 2, j],
                        Alu.add,
                    )
                    if j == NJ - 1:
                        inst.then_inc(vdone, 1)
                    else:
                        inst.then_inc(vch, 1)
                        nvo += 1

            for t4 in range(NCH):  # peeled rep 0
                dve_chunk(t4, 16 * (t4 // 2 + 1),
                          16 * (t4 - 1) if t4 >= 2 else None,
                          VPC * t4, NJ * t4)
            with vector.Fori(1, n_reps) as i:
                for t4 in range(NCH):
                    dve_chunk(t4, i * 32 + 16 * (t4 // 2) + 16,
                              i * (16 * NCH) + 16 * t4 - 16,
                              i * (VPC * NCH) + VPC * t4,
                              i * (NJ * NCH) + NJ * t4)

        @block.sync
        def _(sync):
            for t4 in range(NCH):  # peeled rep 0
                sync.wait_ge(vdone, t4 + 1)
                sync.dma_start(y.ap()[t4], o_sb[:, t4 % 2]).then_inc(ssem, 16)
            with sync.Fori(1, n_reps) as i:
                for t4 in range(NCH):
                    sync.wait_ge(vdone, i * NCH + t4 + 1)
                    sync.dma_start(
                        y.ap()[t4], o_sb[:, t4 % 2]
                    ).then_inc(ssem, 16)
            sync.wait_ge(ssem, 16 * NCH * n_reps)

    nc.compile()
    return nc


def _prep_host(x, weights, idx_a, idx_b):
    x = np.asarray(x, dtype=np.float32)
    w = np.asarray(weights, dtype=np.float32)
    e = np.exp(w - w.max(axis=1, keepdims=True))
    sm = e / e.sum(axis=1, keepdims=True)
    coeffs = (sm @ GATE_COEFFS).astype(np.float32)           # [8192, 4]
    xt = np.ascontiguousarray(x.T).astype(np.float16)        # [8192, 4096]
    ia = np.asarray(idx_a).astype(np.int16)
    ib = np.asarray(idx_b).astype(np.int16)

    IC = CHUNK // 16
    idxws, coefws = [], []
    for c in range(N_CORES):
        iw = np.zeros((16, NCH * 2 * IC), np.int16)
        cw = np.zeros((128, NCH * NJ * 4), np.float32)
        for t4 in range(NCH):
            base = c * OC + t4 * CHUNK
            for w_i, seq in ((0, ia), (1, ib)):
                vals = seq[base:base + CHUNK]
                # wrapped: idx_sb[i % 16, i // 16] = vals[i]
                m = vals.reshape(IC, 16).T                    # [16, IC]
                iw[:, (t4 * 2 + w_i) * IC:(t4 * 2 + w_i + 1) * IC] = m
            for j in range(NJ):
                cols = coeffs[base + j * 128: base + (j + 1) * 128]  # [128,4]
                cw[:, (t4 * NJ + j) * 4:(t4 * NJ + j) * 4 + 4] = cols
        idxws.append(np.tile(iw, (8, 1)))
        coefws.append(cw)
    return xt, idxws, coefws


def _in_maps(x, weights, idx_a, idx_b):
    xt, idxws, coefws = _prep_host(x, weights, idx_a, idx_b)
    return [{"xt": xt, "idxw": idxws[c], "coefw": coefws[c]}
            for c in range(N_CORES)]


def kernel(x, weights, idx_a, idx_b):
    from concourse.bass_utils import run_bass_kernel_spmd

    in_maps = _in_maps(x, weights, idx_a, idx_b)
    if "nc" not in _CACHE:
        _CACHE["nc"] = _build_nc()
    nc = _CACHE["nc"]
    res = run_bass_kernel_spmd(nc, in_maps, list(range(N_CORES)))
    parts = []
    for c in range(N_CORES):
        yc = res.results[c]["y"]                 # [NCH, 128, NJ, BATCH] f16
        parts.append(yc.transpose(0, 2, 1, 3).reshape(OC, BATCH))
    out = np.concatenate(parts, axis=0)          # [OUT_DIM, BATCH]
    return np.ascontiguousarray(out.T).astype(np.float32)
